# revision 24
# baseline (speedup 1.0000x reference)
"""Trainium2 Bass kernel for nn_NestedFormula (basis-function formulation).

Tree: DEPTH=4, V=4. Level sizes n4=1, n3=5, n2=25, n1=125, n0=125.
  f1[n] = sum_v lam1[n,v] * x_v^pow1[n,v] + lam0[n]
  fd[n] = sum_v lamd[n,v] * x_v^powd[n,v] * f_{d-1}[5n+v] + f_{d-1}[5n+4]
  out   = f4[0]                          (per batch element)

Key idea: on x in [0.5, 1.5], x^p == sum_k c_k(p) * x^{a_k} for 8 shared
basis exponents a_k (a_0 = 0), fit error ~1e-9, |c| <= ~2.  Level 1 (500
exps in the reference = the baseline bottleneck) collapses into one K=32
matmul per chunk against the feature tile P, gathered straight into
level-2 order.  E2/E34 remain scalar-engine exps because elementwise ops
can read at most one PSUM operand, so the E-side must live in SBUF.

Engine budget per core (16384 batch):
  PE:     mmA (f1 gather) 16.4k cols + mmC (f2 gather) 16.4k + G3/G4 8.2k
  Scalar: ln (packed 128x512) + P exp + E2 exp + E34 exp + psy copy
  DVE:    X2 = e2 * ps1, X3 = e34 * f2g, X4 = e34 * ps3  (1024-wide)
  GpSimd: issues the replication DMAs (idle otherwise)

Data layout: batch as 4 chunks of 4096 on partition bands 32c.
  P   [128, 4096] rows 32c+4k+v = x_v^{a_k} of chunk c
  e2s [128, 4096] per chunk: rows 4n+v = x_v^pow2[n,v], rows 100+: 1
  e34 [128, 4096] rows 32c+m2 = E3 (m2<20), rows 32c+25+u = E4, else 1
  t is ln x: computed packed [128, 512] (rows 32v+cb), bounced to DRAM,
  broadcast-read into the three replicated layouts above.
"""
import numpy as np

import concourse.bacc as bacc
import concourse.mybir as mybir
from concourse.tile import TileContext

DEPTH = 4
V = 4
B = 131072
M_CORES = 8
BS = B // M_CORES          # 16384 per core
CHUNK = 4096
NCH = BS // CHUNK          # 4
MMN = 512                  # matmul free dim (one PSUM bank)
BP = 1024                  # elementwise block width
NBP = CHUNK // BP          # 4

F32 = mybir.dt.float32
F32R = mybir.dt.float32r

KB = 8                     # basis size (a_0 = 0)


def _sigma1(m):
    # ps1 row m -> level-1 node index (f1 gather)
    if m < 100:
        return 5 * (m // 4) + (m % 4)
    return 5 * (m - 100) + 4


def _tau2(m):
    # f2g row m -> level-2 node index (f2 gather)
    if m < 20:
        return 5 * (m // 4) + (m % 4)
    return 5 * (m - 20) + 4


def _fit_basis(pows_all):
    pmin, pmax = pows_all.min() - 0.1, pows_all.max() + 0.1
    k = np.arange(KB - 1)
    anz = 0.5 * (pmin + pmax) + 0.5 * (pmax - pmin) * np.cos(
        np.pi * (2 * k + 1) / (2 * (KB - 1)))
    a = np.concatenate([[0.0], np.sort(anz)])
    ngrid = 257
    s = np.cos(np.pi * (2 * np.arange(ngrid) + 1) / (2 * ngrid))
    xg = 1.0 + 0.5 * s
    A = xg[:, None] ** a[None, :]

    def coeffs(p):
        T = xg[:, None] ** p.ravel()[None, :]
        C, *_ = np.linalg.lstsq(A, T, rcond=None)
        return C.T.reshape(p.shape + (KB,)).astype(np.float64)

    return a, coeffs


def build_constants(lam0, lam1, pow1, lam2, pow2, lam3, pow3, lam4, pow4):
    lam0, lam1, lam2, lam3, lam4 = [
        np.asarray(z, np.float64) for z in (lam0, lam1, lam2, lam3, lam4)]
    pow2, pow3, pow4 = [np.asarray(p, np.float64) for p in (pow2, pow3, pow4)]
    a, coeffs = _fit_basis(np.asarray(pow1).ravel())
    C1 = coeffs(np.asarray(pow1))      # (125, 4, KB)

    c = {}
    # avec: per-partition exponent for the P-build Exp (rows 32c+4k+v -> a_k)
    avec = np.zeros((128, 1), np.float32)
    for cc in range(NCH):
        for k in range(KB):
            for v in range(V):
                avec[32 * cc + 4 * k + v, 0] = a[k]
    c["avec"] = avec

    # q2vec: E2 exp scales, rows 4n+v = pow2[n,v], rows 100+ -> 0 (E2=1)
    q2vec = np.zeros((128, 1), np.float32)
    for n in range(25):
        for v in range(V):
            q2vec[4 * n + v, 0] = pow2[n, v]
    c["q2vec"] = q2vec

    # q34vec: rows 32c+m2 -> pow3 (m2<20), rows 32c+25+u -> pow4, else 0
    q34vec = np.zeros((128, 1), np.float32)
    for cc in range(NCH):
        r0 = 32 * cc
        for m2 in range(20):
            q34vec[r0 + m2, 0] = pow3[m2 // 4, m2 % 4]
        for u in range(4):
            q34vec[r0 + 25 + u, 0] = pow4[0, u]
    c["q34vec"] = q34vec

    # W1g [32, 128]: col m = gathered f1 (sigma1); rows 4k+v = feature (k,v)
    w1g = np.zeros((32, 128), np.float64)
    for m in range(125):
        n = _sigma1(m)
        for v in range(V):
            for k in range(KB):
                w1g[4 * k + v, m] += lam1[n, v] * C1[n, v, k]
        w1g[0, m] += lam0[n]           # feature (k=0,v=0) == 1
    c["w1g"] = w1g.astype(np.float32)
    # chunk-3 variant: full-K lhsT, weights at rows 96..127 (PE tile base
    # 96 is not encodable)
    w1g3 = np.zeros((128, 128), np.float64)
    w1g3[96:128, :] = w1g
    c["w1g3"] = w1g3.astype(np.float32)

    # G2l per chunk: [128, 4*128], chunk c cols 128c+(32c+j) = f2 gather
    g2l = np.zeros((128, 32), np.float64)
    for m2 in range(25):
        n2t = _tau2(m2)
        for v in range(V):
            g2l[4 * n2t + v, m2] = lam2[n2t, v]
        g2l[100 + n2t, m2] = 1.0       # + f1[5*n2t+4]
    g2l4 = np.zeros((128, 4 * 128), np.float64)
    for cc in range(NCH):
        g2l4[:, 128 * cc + 32 * cc: 128 * cc + 32 * cc + 32] = g2l
    c["g2l4"] = g2l4.astype(np.float32)

    # G3l [128, 128] block-diag: col 32c+25+n3 = f3[n3] of chunk c
    g3l = np.zeros((128, 128), np.float64)
    for cc in range(NCH):
        r0 = 32 * cc
        for n3 in range(5):
            for v in range(V):
                g3l[r0 + 4 * n3 + v, r0 + 25 + n3] = lam3[n3, v]
            g3l[r0 + 20 + n3, r0 + 25 + n3] = 1.0
    c["g3l"] = g3l.astype(np.float32)

    # G4l [128, 4]: col c = y of chunk c
    g4l = np.zeros((128, 4), np.float64)
    for cc in range(NCH):
        r0 = 32 * cc
        for u in range(4):
            g4l[r0 + 25 + u, cc] = lam4[0, u]
        g4l[r0 + 29, cc] = 1.0
    c["g4l"] = g4l.astype(np.float32)
    return c


def build_bass():
    nc = bacc.Bacc()
    xt = nc.dram_tensor("xt", (V, BS), F32, kind="ExternalInput")
    avec = nc.dram_tensor("avec", (128, 1), F32, kind="ExternalInput")
    q2vec = nc.dram_tensor("q2vec", (128, 1), F32, kind="ExternalInput")
    q34vec = nc.dram_tensor("q34vec", (128, 1), F32, kind="ExternalInput")
    w1g = nc.dram_tensor("w1g", (32, 128), F32R, kind="ExternalInput")
    w1g3 = nc.dram_tensor("w1g3", (128, 128), F32R, kind="ExternalInput")
    g2l4 = nc.dram_tensor("g2l4", (128, 4 * 128), F32R, kind="ExternalInput")
    g3l = nc.dram_tensor("g3l", (128, 128), F32R, kind="ExternalInput")
    g4l = nc.dram_tensor("g4l", (128, 4), F32R, kind="ExternalInput")
    y = nc.dram_tensor("y", (BS,), F32, kind="ExternalOutput")

    EXP = mybir.ActivationFunctionType.Exp
    LN = mybir.ActivationFunctionType.Ln
    COPY = mybir.ActivationFunctionType.Copy

    with TileContext(nc) as tc:
        with tc.tile_pool(name="const", bufs=1) as cpool, \
             tc.tile_pool(name="sb", bufs=1) as spool, \
             tc.tile_pool(name="dram", bufs=1, space="DRAM") as dpool, \
             tc.tile_pool(name="psA", bufs=1, space="PSUM") as ppA, \
             tc.tile_pool(name="psB", bufs=1, space="PSUM") as ppB:

            # ---------- constants ----------
            av = cpool.tile([128, 1], F32)
            nc.sync.dma_start(out=av[:], in_=avec[:, :])
            q2v = cpool.tile([128, 1], F32, tag="q2v")
            nc.sync.dma_start(out=q2v[:], in_=q2vec[:, :])
            q34v = cpool.tile([128, 1], F32, tag="q34v")
            nc.sync.dma_start(out=q34v[:], in_=q34vec[:, :])

            def load_c(dt, shape, tag):
                r = cpool.tile(list(shape), F32R, tag=tag)
                nc.sync.dma_start(out=r[:], in_=dt[:, :])
                return r

            # w1g replicated to bands 0/32/64 (lhsT base == rhs base rule)
            w1gt = cpool.tile([128, 128], F32R, tag="w1g")
            nc.sync.dma_start(
                out=w1gt[:96, :],
                in_=w1g[:, :].unsqueeze(0).broadcast_to([3, 32, 128]))
            w1gt3 = load_c(w1g3, (128, 128), "w1g3")
            g2lt = load_c(g2l4, (128, 4 * 128), "g2l4")
            g3lt = load_c(g3l, (128, 128), "g3l")
            g4lt = load_c(g4l, (128, 4), "g4l")

            # ---------- t = ln x, packed [128, 512]: row 32v+cb ----------
            xpk = cpool.tile([128, MMN], F32, tag="xpk")
            nc.sync.dma_start(
                out=xpk[:],
                in_=xt[:, :].rearrange("v (b j) -> v b j", j=MMN))
            nc.scalar.activation(xpk[:], xpk[:], LN)
            tpd = dpool.tile([128, MMN], F32, tag="tpd")   # DRAM bounce
            nc.sync.dma_start(out=tpd[:, :], in_=xpk[:])

            # tpd[32v+8c+q, j] = t_v[chunk c, col 512q+j]
            def tvc(cc):
                return tpd[:, :].rearrange("(v b) j -> v b j", b=32)[
                    :, 8 * cc:8 * cc + 8, :]

            # ---------- replicated layouts + exps ----------
            P = cpool.tile([128, CHUNK], F32R, tag="P")
            trp = cpool.tile([128, CHUNK], F32, tag="trp")
            t34 = cpool.tile([128, CHUNK], F32, tag="t34")
            t2s = [cpool.tile([128, CHUNK], F32, tag=f"t2_{cc}",
                              name=f"t2_{cc}")
                   for cc in range(NCH)]
            # exps run in-place over their t inputs (DVE-only consumers)
            e34 = t34
            e2s = t2s

            for cc in range(NCH):
                # trp rows 32c+4k+v = t_v chunk c  (P input)
                nc.sync.dma_start(
                    out=trp[32 * cc:32 * cc + 32, :],
                    in_=tvc(cc).unsqueeze(0)
                    .broadcast_to([KB, V, 8, MMN]))
                # t34 rows 32c+m2 <- t_{m2%4} (m2<20); filler; rows 25..28
                # overwritten with t_u
                nc.sync.dma_start(
                    out=t34[32 * cc:32 * cc + 20, :],
                    in_=tvc(cc).unsqueeze(0)
                    .broadcast_to([5, V, 8, MMN]))
                nc.sync.dma_start(
                    out=t34[32 * cc + 20:32 * cc + 32, :],
                    in_=tvc(cc).unsqueeze(0)
                    .broadcast_to([3, V, 8, MMN]))
                nc.sync.dma_start(
                    out=t34[32 * cc + 25:32 * cc + 29, :],
                    in_=tvc(cc))
                # t2_c rows 4n+v = t_v chunk c (32x)
                nc.sync.dma_start(
                    out=t2s[cc][:, :],
                    in_=tvc(cc).unsqueeze(0)
                    .broadcast_to([32, V, 8, MMN]))
                nc.scalar.activation(t2s[cc][:].bitcast(F32R), t2s[cc][:],
                                     EXP, scale=q2v[:, 0:1])
            nc.scalar.activation(P[:, 0:2048], trp[:, 0:2048], EXP,
                                 scale=av[:, 0:1])
            nc.scalar.activation(P[:, 2048:4096], trp[:, 2048:4096], EXP,
                                 scale=av[:, 0:1])
            nc.scalar.activation(t34[:, 0:2048].bitcast(F32R),
                                 t34[:, 0:2048], EXP, scale=q34v[:, 0:1])
            nc.scalar.activation(t34[:, 2048:4096].bitcast(F32R),
                                 t34[:, 2048:4096], EXP,
                                 scale=q34v[:, 0:1])

            # ---------- main loop over 1024-col blocks ----------
            for bp in range(NBP):
                b0 = bp * BP
                x2s = []
                for cc in range(NCH):
                    r0 = 32 * cc
                    ps1 = ppA.tile([128, BP], F32, tag="ps1", bufs=2)
                    for s in range(BP // MMN):
                        sl = slice(b0 + s * MMN, b0 + (s + 1) * MMN)
                        if cc < 3:
                            nc.tensor.matmul(ps1[:, s * MMN:(s + 1) * MMN],
                                             w1gt[r0:r0 + 32, :],
                                             P[r0:r0 + 32, sl],
                                             start=True, stop=True)
                        else:
                            nc.tensor.matmul(ps1[:, s * MMN:(s + 1) * MMN],
                                             w1gt3[:], P[:, sl],
                                             start=True, stop=True)
                    x2 = spool.tile([128, BP], F32R, tag=f"x2_{cc}",
                                    name=f"x2_{cc}", bufs=2)
                    nc.vector.tensor_mul(x2[:],
                                         e2s[cc][:, b0:b0 + BP]
                                         .bitcast(F32R),
                                         ps1[:].bitcast(F32R))
                    x2s.append(x2)
                f2g = ppA.tile([128, BP], F32, tag="f2g", bufs=1)
                for s in range(BP // MMN):
                    for cc in range(NCH):
                        nc.tensor.matmul(
                            f2g[:, s * MMN:(s + 1) * MMN],
                            g2lt[:, 128 * cc:128 * cc + 128],
                            x2s[cc][:, s * MMN:(s + 1) * MMN],
                            start=(cc == 0), stop=(cc == 3))
                x3 = spool.tile([128, BP], F32R, tag="x3", bufs=2)
                nc.vector.tensor_mul(x3[:],
                                     e34[:, b0:b0 + BP].bitcast(F32R),
                                     f2g[:].bitcast(F32R))
                ysb = spool.tile([4, BP], F32, tag="ysb", bufs=2)
                for s in range(BP // MMN):
                    ss = slice(s * MMN, (s + 1) * MMN)
                    ps3 = ppB.tile([128, MMN], F32, tag="ps3", bufs=1)
                    nc.tensor.matmul(ps3[:], g3lt[:], x3[:, ss],
                                     start=True, stop=True)
                    x4 = spool.tile([128, MMN], F32R, tag="x4", bufs=2)
                    nc.vector.tensor_mul(
                        x4[:], e34[:, b0 + s * MMN:b0 + (s + 1) * MMN]
                        .bitcast(F32R), ps3[:].bitcast(F32R))
                    psy = ppB.tile([4, MMN], F32, tag="psy", bufs=1)
                    nc.tensor.matmul(psy[:], g4lt[:], x4[:],
                                     start=True, stop=True)
                    nc.scalar.activation(ysb[:, ss], psy[:], COPY)
                nc.sync.dma_start(
                    out=y[:].rearrange("(c i) -> c i", i=CHUNK)[:, b0:b0 + BP],
                    in_=ysb[:])

    nc.compile()
    return nc


def kernel(x, lam0, lam1, pow1, lam2, pow2, lam3, pow3, lam4, pow4):
    x = np.asarray(x, np.float32)
    consts = build_constants(lam0, lam1, pow1, lam2, pow2,
                             lam3, pow3, lam4, pow4)
    nc = build_bass()

    in_maps = []
    for k in range(M_CORES):
        shard = x[k * BS:(k + 1) * BS, :]
        m = {"xt": np.ascontiguousarray(shard.T)}
        m.update(consts)
        in_maps.append(m)

    from concourse.bass_utils import run_bass_kernel_spmd
    res = run_bass_kernel_spmd(nc, in_maps, list(range(M_CORES)))
    out = np.concatenate([res.results[k]["y"] for k in range(M_CORES)])
    return out[:, None].astype(np.float32)


if __name__ == "__main__":
    import reference
    inputs = {k: np.asarray(v) for k, v in reference.setup_inputs().items()}
    got = kernel(**inputs)
    exp = np.asarray(reference.reference(**inputs))
    err = np.abs(got - exp).max() / (np.abs(exp).max() + 1e-30)
    print("shape", got.shape, "relerr", err)


# revision 27
# speedup vs baseline: 1.6029x; 1.6029x over previous
"""Trainium2 Bass kernel for nn_NestedFormula (basis-function formulation).

Tree: DEPTH=4, V=4. Level sizes n4=1, n3=5, n2=25, n1=125, n0=125.
  f1[n] = sum_v lam1[n,v] * x_v^pow1[n,v] + lam0[n]
  fd[n] = sum_v lamd[n,v] * x_v^powd[n,v] * f_{d-1}[5n+v] + f_{d-1}[5n+4]
  out   = f4[0]                          (per batch element)

Key idea: on x in [0.5, 1.5], x^p == sum_k c_k(p) * x^{a_k} for 8 shared
basis exponents a_k (a_0 = 0), fit error ~1e-9, |c| <= ~2.  Level 1 (500
exps in the reference = the baseline bottleneck) collapses into one K=32
matmul per chunk against the feature tile P, gathered straight into
level-2 order.  E2/E34 remain scalar-engine exps because elementwise ops
can read at most one PSUM operand, so the E-side must live in SBUF.

Engine budget per core (16384 batch):
  PE:     mmA (f1 gather) 16.4k cols + mmC (f2 gather) 16.4k + G3/G4 8.2k
  Scalar: ln (packed 128x512) + P exp + E2 exp + E34 exp + psy copy
  DVE:    X2 = e2 * ps1, X3 = e34 * f2g, X4 = e34 * ps3  (1024-wide)
  GpSimd: issues the replication DMAs (idle otherwise)

Data layout: batch as 4 chunks of 4096 on partition bands 32c.
  P   [128, 4096] rows 32c+4k+v = x_v^{a_k} of chunk c
  e2s [128, 4096] per chunk: rows 4n+v = x_v^pow2[n,v], rows 100+: 1
  e34 [128, 4096] rows 32c+m2 = E3 (m2<20), rows 32c+25+u = E4, else 1
  t is ln x: computed packed [128, 512] (rows 32v+cb), bounced to DRAM,
  broadcast-read into the three replicated layouts above.
"""
import numpy as np

import concourse.bacc as bacc
import concourse.mybir as mybir
from concourse.tile import TileContext

DEPTH = 4
V = 4
B = 131072
M_CORES = 8
BS = B // M_CORES          # 16384 per core
CHUNK = 4096
NCH = BS // CHUNK          # 4
MMN = 512                  # matmul free dim (one PSUM bank)
BP = 1024                  # elementwise block width
NBP = CHUNK // BP          # 4

F32 = mybir.dt.float32
F32R = mybir.dt.float32r

KB = 8                     # basis size (a_0 = 0)


def _sigma1(m):
    # ps1 row m -> level-1 node index (f1 gather)
    if m < 100:
        return 5 * (m // 4) + (m % 4)
    return 5 * (m - 100) + 4


def _tau2(m):
    # f2g row m -> level-2 node index (f2 gather)
    if m < 20:
        return 5 * (m // 4) + (m % 4)
    return 5 * (m - 20) + 4


def _fit_basis(pows_all):
    pmin, pmax = pows_all.min() - 0.1, pows_all.max() + 0.1
    k = np.arange(KB - 1)
    anz = 0.5 * (pmin + pmax) + 0.5 * (pmax - pmin) * np.cos(
        np.pi * (2 * k + 1) / (2 * (KB - 1)))
    a = np.concatenate([[0.0], np.sort(anz)])
    ngrid = 257
    s = np.cos(np.pi * (2 * np.arange(ngrid) + 1) / (2 * ngrid))
    xg = 1.0 + 0.5 * s
    A = xg[:, None] ** a[None, :]

    def coeffs(p):
        T = xg[:, None] ** p.ravel()[None, :]
        C, *_ = np.linalg.lstsq(A, T, rcond=None)
        return C.T.reshape(p.shape + (KB,)).astype(np.float64)

    return a, coeffs


def build_constants(lam0, lam1, pow1, lam2, pow2, lam3, pow3, lam4, pow4):
    lam0, lam1, lam2, lam3, lam4 = [
        np.asarray(z, np.float64) for z in (lam0, lam1, lam2, lam3, lam4)]
    pow2, pow3, pow4 = [np.asarray(p, np.float64) for p in (pow2, pow3, pow4)]
    a, coeffs = _fit_basis(np.asarray(pow1).ravel())
    C1 = coeffs(np.asarray(pow1))      # (125, 4, KB)

    c = {}
    # avec: per-partition exponent for the P-build Exp (rows 32c+4k+v -> a_k)
    avec = np.zeros((128, 1), np.float32)
    for cc in range(NCH):
        for k in range(KB):
            for v in range(V):
                avec[32 * cc + 4 * k + v, 0] = a[k]
    c["avec"] = avec

    # q2vec: E2 exp scales, rows 4n+v = pow2[n,v], rows 100+ -> 0 (E2=1)
    q2vec = np.zeros((128, 1), np.float32)
    for n in range(25):
        for v in range(V):
            q2vec[4 * n + v, 0] = pow2[n, v]
    c["q2vec"] = q2vec

    # q34vec (v = row%4 everywhere, matching trp): rows 32c+m2 -> pow3
    # (m2<20), rows 32c+20+u -> pow4 (v=u matches (20+u)%4), else 0
    q34vec = np.zeros((128, 1), np.float32)
    for cc in range(NCH):
        r0 = 32 * cc
        for m2 in range(20):
            q34vec[r0 + m2, 0] = pow3[m2 // 4, m2 % 4]
        for u in range(4):
            q34vec[r0 + 20 + u, 0] = pow4[0, u]
    c["q34vec"] = q34vec

    # W1g [32, 128]: col m = gathered f1 (sigma1); rows 4k+v = feature (k,v)
    w1g = np.zeros((32, 128), np.float64)
    for m in range(125):
        n = _sigma1(m)
        for v in range(V):
            for k in range(KB):
                w1g[4 * k + v, m] += lam1[n, v] * C1[n, v, k]
        w1g[0, m] += lam0[n]           # feature (k=0,v=0) == 1
    c["w1g"] = w1g.astype(np.float32)
    # chunk-3 variant: full-K lhsT, weights at rows 96..127 (PE tile base
    # 96 is not encodable)
    w1g3 = np.zeros((128, 128), np.float64)
    w1g3[96:128, :] = w1g
    c["w1g3"] = w1g3.astype(np.float32)

    # G2l per chunk: [128, 4*128], chunk c cols 128c+(32c+j) = f2 gather
    g2l = np.zeros((128, 32), np.float64)
    for m2 in range(25):
        n2t = _tau2(m2)
        col = m2 if m2 < 20 else 5 + m2      # pass cols at 25..29
        for v in range(V):
            g2l[4 * n2t + v, col] = lam2[n2t, v]
        g2l[100 + n2t, col] = 1.0      # + f1[5*n2t+4]
    g2l4 = np.zeros((128, 4 * 128), np.float64)
    for cc in range(NCH):
        g2l4[:, 128 * cc + 32 * cc: 128 * cc + 32 * cc + 32] = g2l
    c["g2l4"] = g2l4.astype(np.float32)

    # G3l [128, 128] block-diag: col 32c+20+n3 = f3[n3] of chunk c
    # (X3 pass rows live at 32c+25+n3; E4 rows at 32c+20+u)
    g3l = np.zeros((128, 128), np.float64)
    for cc in range(NCH):
        r0 = 32 * cc
        for n3 in range(5):
            for v in range(V):
                g3l[r0 + 4 * n3 + v, r0 + 20 + n3] = lam3[n3, v]
            g3l[r0 + 25 + n3, r0 + 20 + n3] = 1.0
    c["g3l"] = g3l.astype(np.float32)

    # G4l [128, 4]: col c = y of chunk c (X4 rows 32c+20+u; f3[4] at +24)
    g4l = np.zeros((128, 4), np.float64)
    for cc in range(NCH):
        r0 = 32 * cc
        for u in range(4):
            g4l[r0 + 20 + u, cc] = lam4[0, u]
        g4l[r0 + 24, cc] = 1.0
    c["g4l"] = g4l.astype(np.float32)
    return c


def build_bass():
    nc = bacc.Bacc()
    xt = nc.dram_tensor("xt", (V, BS), F32, kind="ExternalInput")
    avec = nc.dram_tensor("avec", (128, 1), F32, kind="ExternalInput")
    q2vec = nc.dram_tensor("q2vec", (128, 1), F32, kind="ExternalInput")
    q34vec = nc.dram_tensor("q34vec", (128, 1), F32, kind="ExternalInput")
    w1g = nc.dram_tensor("w1g", (32, 128), F32R, kind="ExternalInput")
    w1g3 = nc.dram_tensor("w1g3", (128, 128), F32R, kind="ExternalInput")
    g2l4 = nc.dram_tensor("g2l4", (128, 4 * 128), F32R, kind="ExternalInput")
    g3l = nc.dram_tensor("g3l", (128, 128), F32R, kind="ExternalInput")
    g4l = nc.dram_tensor("g4l", (128, 4), F32R, kind="ExternalInput")
    y = nc.dram_tensor("y", (BS,), F32, kind="ExternalOutput")

    EXP = mybir.ActivationFunctionType.Exp
    LN = mybir.ActivationFunctionType.Ln
    COPY = mybir.ActivationFunctionType.Copy

    with TileContext(nc) as tc:
        with tc.tile_pool(name="const", bufs=1) as cpool, \
             tc.tile_pool(name="sb", bufs=1) as spool, \
             tc.tile_pool(name="dram", bufs=1, space="DRAM") as dpool, \
             tc.tile_pool(name="psA", bufs=1, space="PSUM") as ppA, \
             tc.tile_pool(name="psB", bufs=1, space="PSUM") as ppB:

            # ---------- constants ----------
            av = cpool.tile([128, 1], F32)
            nc.sync.dma_start(out=av[:], in_=avec[:, :])
            q2v = cpool.tile([128, 1], F32, tag="q2v")
            nc.sync.dma_start(out=q2v[:], in_=q2vec[:, :])
            q34v = cpool.tile([128, 1], F32, tag="q34v")
            nc.sync.dma_start(out=q34v[:], in_=q34vec[:, :])

            def load_c(dt, shape, tag):
                r = cpool.tile(list(shape), F32R, tag=tag)
                nc.sync.dma_start(out=r[:], in_=dt[:, :])
                return r

            # w1g replicated to bands 0/32/64 (lhsT base == rhs base rule)
            w1gt = cpool.tile([128, 128], F32R, tag="w1g")
            nc.sync.dma_start(
                out=w1gt[:96, :],
                in_=w1g[:, :].unsqueeze(0).broadcast_to([3, 32, 128]))
            w1gt3 = load_c(w1g3, (128, 128), "w1g3")
            g2lt = load_c(g2l4, (128, 4 * 128), "g2l4")
            g3lt = load_c(g3l, (128, 128), "g3l")
            g4lt = load_c(g4l, (128, 4), "g4l")

            # ---------- t = ln x, packed [128, 512]: row 32v+cb ----------
            xpk = cpool.tile([128, MMN], F32, tag="xpk")
            nc.sync.dma_start(
                out=xpk[:],
                in_=xt[:, :].rearrange("v (b j) -> v b j", j=MMN))
            nc.scalar.activation(xpk[:], xpk[:], LN)
            tpd = dpool.tile([128, MMN], F32, tag="tpd")   # DRAM bounce
            nc.sync.dma_start(out=tpd[:, :], in_=xpk[:])

            # tpd[32v+8c+q, j] = t_v[chunk c, col 512q+j]
            def tvc(cc):
                return tpd[:, :].rearrange("(v b) j -> v b j", b=32)[
                    :, 8 * cc:8 * cc + 8, :]

            # ---------- replicated layouts + exps ----------
            P = cpool.tile([128, CHUNK], F32R, tag="P")
            trp = cpool.tile([128, CHUNK], F32, tag="trp")
            e34 = cpool.tile([128, CHUNK], F32R, tag="e34")
            t2s = [cpool.tile([128, CHUNK], F32, tag=f"t2_{cc}",
                              name=f"t2_{cc}")
                   for cc in range(NCH)]
            e2s = t2s                  # E2 exp runs in place

            HW_ = CHUNK // 2
            # replication DMAs spread across the four DMA queues
            # (queue = issuing engine); column halves for finer pipelining
            qeng = [nc.sync, nc.scalar, nc.gpsimd, nc.sync]
            for h in range(2):
                h0 = h * HW_
                for cc in range(NCH):
                    qeng[cc].dma_start(
                        out=trp[32 * cc:32 * cc + 32, h0:h0 + HW_],
                        in_=tvc(cc)[:, 4 * h:4 * h + 4, :].unsqueeze(0)
                        .broadcast_to([KB, V, 4, MMN]))
                for cc in range(NCH):
                    # t2_c rows 4n+v = t_v of chunk c (32x replication)
                    qeng[cc].dma_start(
                        out=t2s[cc][:, h0:h0 + HW_],
                        in_=tvc(cc)[:, 4 * h:4 * h + 4, :].unsqueeze(0)
                        .broadcast_to([32, V, 4, MMN]))
                nc.scalar.activation(P[:, h0:h0 + HW_], trp[:, h0:h0 + HW_],
                                     EXP, scale=av[:, 0:1])
                nc.scalar.activation(e2s[0][:, h0:h0 + HW_].bitcast(F32R),
                                     t2s[0][:, h0:h0 + HW_], EXP,
                                     scale=q2v[:, 0:1])
                nc.scalar.activation(e34[:, h0:h0 + HW_],
                                     trp[:, h0:h0 + HW_], EXP,
                                     scale=q34v[:, 0:1])
                for cc in range(1, NCH):
                    nc.scalar.activation(e2s[cc][:, h0:h0 + HW_]
                                         .bitcast(F32R),
                                         t2s[cc][:, h0:h0 + HW_], EXP,
                                         scale=q2v[:, 0:1])

            # ---------- main loop over 1024-col blocks ----------
            for bp in range(NBP):
                b0 = bp * BP
                x2s = []
                for cc in range(NCH):
                    r0 = 32 * cc
                    ps1 = ppA.tile([128, BP], F32, tag="ps1", bufs=2)
                    for s in range(BP // MMN):
                        sl = slice(b0 + s * MMN, b0 + (s + 1) * MMN)
                        if cc < 3:
                            nc.tensor.matmul(ps1[:, s * MMN:(s + 1) * MMN],
                                             w1gt[r0:r0 + 32, :],
                                             P[r0:r0 + 32, sl],
                                             start=True, stop=True)
                        else:
                            nc.tensor.matmul(ps1[:, s * MMN:(s + 1) * MMN],
                                             w1gt3[:], P[:, sl],
                                             start=True, stop=True)
                    x2 = spool.tile([128, BP], F32R, tag=f"x2_{cc}",
                                    name=f"x2_{cc}", bufs=2)
                    nc.vector.tensor_mul(x2[:],
                                         e2s[cc][:, b0:b0 + BP]
                                         .bitcast(F32R),
                                         ps1[:].bitcast(F32R))
                    x2s.append(x2)
                f2g = ppA.tile([128, BP], F32, tag="f2g", bufs=1)
                for s in range(BP // MMN):
                    for cc in range(NCH):
                        nc.tensor.matmul(
                            f2g[:, s * MMN:(s + 1) * MMN],
                            g2lt[:, 128 * cc:128 * cc + 128],
                            x2s[cc][:, s * MMN:(s + 1) * MMN],
                            start=(cc == 0), stop=(cc == 3))
                x3 = spool.tile([128, BP], F32R, tag="x3", bufs=2)
                nc.vector.tensor_mul(x3[:],
                                     e34[:, b0:b0 + BP].bitcast(F32R),
                                     f2g[:].bitcast(F32R))
                ysb = spool.tile([4, BP], F32, tag="ysb", bufs=2)
                for s in range(BP // MMN):
                    ss = slice(s * MMN, (s + 1) * MMN)
                    ps3 = ppB.tile([128, MMN], F32, tag="ps3", bufs=1)
                    nc.tensor.matmul(ps3[:], g3lt[:], x3[:, ss],
                                     start=True, stop=True)
                    x4 = spool.tile([128, MMN], F32R, tag="x4", bufs=2)
                    nc.vector.tensor_mul(
                        x4[:], e34[:, b0 + s * MMN:b0 + (s + 1) * MMN]
                        .bitcast(F32R), ps3[:].bitcast(F32R))
                    psy = ppB.tile([4, MMN], F32, tag="psy", bufs=1)
                    nc.tensor.matmul(psy[:], g4lt[:], x4[:],
                                     start=True, stop=True)
                    nc.scalar.activation(ysb[:, ss], psy[:], COPY)
                nc.sync.dma_start(
                    out=y[:].rearrange("(c i) -> c i", i=CHUNK)[:, b0:b0 + BP],
                    in_=ysb[:])

    nc.compile()
    return nc


def kernel(x, lam0, lam1, pow1, lam2, pow2, lam3, pow3, lam4, pow4):
    x = np.asarray(x, np.float32)
    consts = build_constants(lam0, lam1, pow1, lam2, pow2,
                             lam3, pow3, lam4, pow4)
    nc = build_bass()

    in_maps = []
    for k in range(M_CORES):
        shard = x[k * BS:(k + 1) * BS, :]
        m = {"xt": np.ascontiguousarray(shard.T)}
        m.update(consts)
        in_maps.append(m)

    from concourse.bass_utils import run_bass_kernel_spmd
    res = run_bass_kernel_spmd(nc, in_maps, list(range(M_CORES)))
    out = np.concatenate([res.results[k]["y"] for k in range(M_CORES)])
    return out[:, None].astype(np.float32)


if __name__ == "__main__":
    import reference
    inputs = {k: np.asarray(v) for k, v in reference.setup_inputs().items()}
    got = kernel(**inputs)
    exp = np.asarray(reference.reference(**inputs))
    err = np.abs(got - exp).max() / (np.abs(exp).max() + 1e-30)
    print("shape", got.shape, "relerr", err)


# revision 30
# speedup vs baseline: 1.7804x; 1.1108x over previous
"""Trainium2 Bass kernel for nn_NestedFormula (basis-function formulation).

Tree: DEPTH=4, V=4. Level sizes n4=1, n3=5, n2=25, n1=125, n0=125.
  f1[n] = sum_v lam1[n,v] * x_v^pow1[n,v] + lam0[n]
  fd[n] = sum_v lamd[n,v] * x_v^powd[n,v] * f_{d-1}[5n+v] + f_{d-1}[5n+4]
  out   = f4[0]                          (per batch element)

Key idea: on x in [0.5, 1.5], x^p == sum_k c_k(p) * x^{a_k} for 8 shared
basis exponents a_k (a_0 = 0), fit error ~1e-9, |c| <= ~2.  Level 1 (500
exps in the reference = the baseline bottleneck) collapses into one K=32
matmul per chunk against the feature tile P, gathered straight into
level-2 order.  E2/E34 remain scalar-engine exps because elementwise ops
can read at most one PSUM operand, so the E-side must live in SBUF.

Engine budget per core (16384 batch):
  PE:     mmA (f1 gather) 16.4k cols + mmC (f2 gather) 16.4k + G3/G4 8.2k
  Scalar: ln (packed 128x512) + P exp + E2 exp + E34 exp + psy copy
  DVE:    X2 = e2 * ps1, X3 = e34 * f2g, X4 = e34 * ps3  (1024-wide)
  GpSimd: issues the replication DMAs (idle otherwise)

Data layout: batch as 4 chunks of 4096 on partition bands 32c.
  P   [128, 4096] rows 32c+4k+v = x_v^{a_k} of chunk c
  e2s [128, 4096] per chunk: rows 4n+v = x_v^pow2[n,v], rows 100+: 1
  e34 [128, 4096] rows 32c+m2 = E3 (m2<20), rows 32c+25+u = E4, else 1
  t is ln x: computed packed [128, 512] (rows 32v+cb), bounced to DRAM,
  broadcast-read into the three replicated layouts above.
"""
import numpy as np

import concourse.bacc as bacc
import concourse.mybir as mybir
from concourse.tile import TileContext

DEPTH = 4
V = 4
B = 131072
M_CORES = 8
BS = B // M_CORES          # 16384 per core
CHUNK = 4096
NCH = BS // CHUNK          # 4
MMN = 512                  # matmul free dim (one PSUM bank)
BP = 1024                  # elementwise block width
NBP = CHUNK // BP          # 4

F32 = mybir.dt.float32
F32R = mybir.dt.float32r
F16 = mybir.dt.float16

KB = 8                     # basis size (a_0 = 0)


def _sigma1(m):
    # ps1 row m -> level-1 node index (f1 gather)
    if m < 100:
        return 5 * (m // 4) + (m % 4)
    return 5 * (m - 100) + 4


def _tau2(m):
    # f2g row m -> level-2 node index (f2 gather)
    if m < 20:
        return 5 * (m // 4) + (m % 4)
    return 5 * (m - 20) + 4


def _fit_basis(pows_all):
    pmin, pmax = pows_all.min() - 0.1, pows_all.max() + 0.1
    k = np.arange(KB - 1)
    anz = 0.5 * (pmin + pmax) + 0.5 * (pmax - pmin) * np.cos(
        np.pi * (2 * k + 1) / (2 * (KB - 1)))
    a = np.concatenate([[0.0], np.sort(anz)])
    ngrid = 257
    s = np.cos(np.pi * (2 * np.arange(ngrid) + 1) / (2 * ngrid))
    xg = 1.0 + 0.5 * s
    A = xg[:, None] ** a[None, :]

    def coeffs(p):
        T = xg[:, None] ** p.ravel()[None, :]
        C, *_ = np.linalg.lstsq(A, T, rcond=None)
        return C.T.reshape(p.shape + (KB,)).astype(np.float64)

    return a, coeffs


def build_constants(lam0, lam1, pow1, lam2, pow2, lam3, pow3, lam4, pow4):
    lam0, lam1, lam2, lam3, lam4 = [
        np.asarray(z, np.float64) for z in (lam0, lam1, lam2, lam3, lam4)]
    pow2, pow3, pow4 = [np.asarray(p, np.float64) for p in (pow2, pow3, pow4)]
    a, coeffs = _fit_basis(np.asarray(pow1).ravel())
    C1 = coeffs(np.asarray(pow1))      # (125, 4, KB)

    c = {}
    # avec: per-partition exponent for the P-build Exp (rows 32c+4k+v -> a_k)
    avec = np.zeros((128, 1), np.float32)
    for cc in range(NCH):
        for k in range(KB):
            for v in range(V):
                avec[32 * cc + 4 * k + v, 0] = a[k]
    c["avec"] = avec

    # q2vec: E2 exp scales, rows 4n+v = pow2[n,v], rows 100+ -> 0 (E2=1)
    q2vec = np.zeros((128, 1), np.float32)
    for n in range(25):
        for v in range(V):
            q2vec[4 * n + v, 0] = pow2[n, v]
    c["q2vec"] = q2vec

    # q34vec (v = row%4 everywhere, matching trp): rows 32c+m2 -> pow3
    # (m2<20), rows 32c+20+u -> pow4 (v=u matches (20+u)%4), else 0
    q34vec = np.zeros((128, 1), np.float32)
    for cc in range(NCH):
        r0 = 32 * cc
        for m2 in range(20):
            q34vec[r0 + m2, 0] = pow3[m2 // 4, m2 % 4]
        for u in range(4):
            q34vec[r0 + 20 + u, 0] = pow4[0, u]
    c["q34vec"] = q34vec

    # W1g [32, 128]: col m = gathered f1 (sigma1); rows 4k+v = feature (k,v)
    w1g = np.zeros((32, 128), np.float64)
    for m in range(125):
        n = _sigma1(m)
        for v in range(V):
            for k in range(KB):
                w1g[4 * k + v, m] += lam1[n, v] * C1[n, v, k]
        w1g[0, m] += lam0[n]           # feature (k=0,v=0) == 1
    c["w1g"] = w1g.astype(np.float32)
    # chunk-3 variant: full-K lhsT, weights at rows 96..127 (PE tile base
    # 96 is not encodable)
    w1g3 = np.zeros((128, 128), np.float64)
    w1g3[96:128, :] = w1g
    c["w1g3"] = w1g3.astype(np.float32)

    # G2l per chunk: [128, 4*128], chunk c cols 128c+(32c+j) = f2 gather
    g2l = np.zeros((128, 32), np.float64)
    for m2 in range(25):
        n2t = _tau2(m2)
        col = m2 if m2 < 20 else 5 + m2      # pass cols at 25..29
        for v in range(V):
            g2l[4 * n2t + v, col] = lam2[n2t, v]
        g2l[100 + n2t, col] = 1.0      # + f1[5*n2t+4]
    g2l4 = np.zeros((128, 4 * 128), np.float64)
    for cc in range(NCH):
        g2l4[:, 128 * cc + 32 * cc: 128 * cc + 32 * cc + 32] = g2l
    c["g2l4"] = g2l4.astype(np.float16)

    # G3l [128, 128] block-diag: col 32c+20+n3 = f3[n3] of chunk c
    # (X3 pass rows live at 32c+25+n3; E4 rows at 32c+20+u)
    g3l = np.zeros((128, 128), np.float64)
    for cc in range(NCH):
        r0 = 32 * cc
        for n3 in range(5):
            for v in range(V):
                g3l[r0 + 4 * n3 + v, r0 + 20 + n3] = lam3[n3, v]
            g3l[r0 + 25 + n3, r0 + 20 + n3] = 1.0
    c["g3l"] = g3l.astype(np.float16)

    # G4l [128, 4]: col c = y of chunk c (X4 rows 32c+20+u; f3[4] at +24)
    g4l = np.zeros((128, 4), np.float64)
    for cc in range(NCH):
        r0 = 32 * cc
        for u in range(4):
            g4l[r0 + 20 + u, cc] = lam4[0, u]
        g4l[r0 + 24, cc] = 1.0
    c["g4l"] = g4l.astype(np.float16)
    return c


def build_bass():
    nc = bacc.Bacc()
    xt = nc.dram_tensor("xt", (V, BS), F32, kind="ExternalInput")
    avec = nc.dram_tensor("avec", (128, 1), F32, kind="ExternalInput")
    q2vec = nc.dram_tensor("q2vec", (128, 1), F32, kind="ExternalInput")
    q34vec = nc.dram_tensor("q34vec", (128, 1), F32, kind="ExternalInput")
    w1g = nc.dram_tensor("w1g", (32, 128), F32R, kind="ExternalInput")
    w1g3 = nc.dram_tensor("w1g3", (128, 128), F32R, kind="ExternalInput")
    g2l4 = nc.dram_tensor("g2l4", (128, 4 * 128), F16, kind="ExternalInput")
    g3l = nc.dram_tensor("g3l", (128, 128), F16, kind="ExternalInput")
    g4l = nc.dram_tensor("g4l", (128, 4), F16, kind="ExternalInput")
    y = nc.dram_tensor("y", (BS,), F32, kind="ExternalOutput")

    EXP = mybir.ActivationFunctionType.Exp
    LN = mybir.ActivationFunctionType.Ln
    COPY = mybir.ActivationFunctionType.Copy

    with TileContext(nc) as tc:
        with tc.tile_pool(name="const", bufs=1) as cpool, \
             tc.tile_pool(name="sb", bufs=1) as spool, \
             tc.tile_pool(name="dram", bufs=1, space="DRAM") as dpool, \
             tc.tile_pool(name="psA", bufs=1, space="PSUM") as ppA, \
             tc.tile_pool(name="psB", bufs=1, space="PSUM") as ppB:

            # ---------- constants ----------
            av = cpool.tile([128, 1], F32)
            nc.sync.dma_start(out=av[:], in_=avec[:, :])
            q2v = cpool.tile([128, 1], F32, tag="q2v")
            nc.sync.dma_start(out=q2v[:], in_=q2vec[:, :])
            q34v = cpool.tile([128, 1], F32, tag="q34v")
            nc.sync.dma_start(out=q34v[:], in_=q34vec[:, :])

            def load_c(dt, shape, tag):
                r = cpool.tile(list(shape), F32R, tag=tag)
                nc.sync.dma_start(out=r[:], in_=dt[:, :])
                return r

            # w1g replicated to bands 0/32/64 (lhsT base == rhs base rule)
            w1gt = cpool.tile([128, 128], F32R, tag="w1g")
            nc.sync.dma_start(
                out=w1gt[:96, :],
                in_=w1g[:, :].unsqueeze(0).broadcast_to([3, 32, 128]))
            w1gt3 = load_c(w1g3, (128, 128), "w1g3")

            def load_h(dt, shape, tag):
                r = cpool.tile(list(shape), F16, tag=tag)
                nc.sync.dma_start(out=r[:], in_=dt[:, :])
                return r

            g2lt = load_h(g2l4, (128, 4 * 128), "g2l4")
            g3lt = load_h(g3l, (128, 128), "g3l")
            g4lt = load_h(g4l, (128, 4), "g4l")

            # ---------- t = ln x, packed [128, 512]: row 32v+cb ----------
            xpk = cpool.tile([128, MMN], F32, tag="xpk")
            nc.sync.dma_start(
                out=xpk[:],
                in_=xt[:, :].rearrange("v (b j) -> v b j", j=MMN))
            xpkh = cpool.tile([128, MMN], F16, tag="xpkh")
            nc.scalar.activation(xpkh[:], xpk[:], LN)
            tpd = dpool.tile([128, MMN], F16, tag="tpd")   # DRAM bounce
            nc.sync.dma_start(out=tpd[:, :], in_=xpkh[:])

            # tpd[32v+8c+q, j] = t_v[chunk c, col 512q+j]
            def tvc(cc):
                return tpd[:, :].rearrange("(v b) j -> v b j", b=32)[
                    :, 8 * cc:8 * cc + 8, :]

            # ---------- replicated layouts + exps (fp16 t data) ----------
            P = cpool.tile([128, CHUNK], F32R, tag="P")
            trp = cpool.tile([128, CHUNK], F16, tag="trp")
            e34 = cpool.tile([128, CHUNK], F16, tag="e34")
            t2s = [cpool.tile([128, CHUNK], F16, tag=f"t2_{cc}",
                              name=f"t2_{cc}")
                   for cc in range(NCH)]
            e2s = t2s                  # E2 exp runs in place (fp16)

            HW_ = CHUNK // 2
            # replication DMAs spread across the three DMA queues
            # (queue = issuing engine); column halves for finer pipelining
            qeng = [nc.sync, nc.scalar, nc.gpsimd]
            qi = 0
            for h in range(2):
                h0 = h * HW_
                for cc in range(NCH):
                    qeng[qi % 3].dma_start(
                        out=trp[32 * cc:32 * cc + 32, h0:h0 + HW_],
                        in_=tvc(cc)[:, 4 * h:4 * h + 4, :].unsqueeze(0)
                        .broadcast_to([KB, V, 4, MMN]))
                    qi += 1
                for cc in range(NCH):
                    # t2_c rows 4n+v = t_v of chunk c (32x replication)
                    qeng[qi % 3].dma_start(
                        out=t2s[cc][:, h0:h0 + HW_],
                        in_=tvc(cc)[:, 4 * h:4 * h + 4, :].unsqueeze(0)
                        .broadcast_to([32, V, 4, MMN]))
                    qi += 1
            for q in range(NBP):
                q0 = q * BP
                nc.scalar.activation(P[:, q0:q0 + BP], trp[:, q0:q0 + BP],
                                     EXP, scale=av[:, 0:1])
                nc.scalar.activation(e34[:, q0:q0 + BP], trp[:, q0:q0 + BP],
                                     EXP, scale=q34v[:, 0:1])
                for cc in range(NCH):
                    nc.scalar.activation(e2s[cc][:, q0:q0 + BP],
                                         t2s[cc][:, q0:q0 + BP], EXP,
                                         scale=q2v[:, 0:1])

            # ---------- main loop over 1024-col blocks ----------
            for bp in range(NBP):
                b0 = bp * BP
                x2s = []
                for cc in range(NCH):
                    r0 = 32 * cc
                    ps1 = ppA.tile([128, BP], F32, tag="ps1", bufs=2)
                    for s in range(BP // MMN):
                        sl = slice(b0 + s * MMN, b0 + (s + 1) * MMN)
                        if cc < 3:
                            nc.tensor.matmul(ps1[:, s * MMN:(s + 1) * MMN],
                                             w1gt[r0:r0 + 32, :],
                                             P[r0:r0 + 32, sl],
                                             start=True, stop=True)
                        else:
                            nc.tensor.matmul(ps1[:, s * MMN:(s + 1) * MMN],
                                             w1gt3[:], P[:, sl],
                                             start=True, stop=True)
                    x2 = spool.tile([128, BP], F16, tag=f"x2_{cc}",
                                    name=f"x2_{cc}", bufs=2)
                    nc.vector.tensor_mul(x2[:], e2s[cc][:, b0:b0 + BP],
                                         ps1[:])
                    x2s.append(x2)
                f2g = ppA.tile([128, BP], F32, tag="f2g", bufs=1)
                for s in range(BP // MMN):
                    for cc in range(NCH):
                        nc.tensor.matmul(
                            f2g[:, s * MMN:(s + 1) * MMN],
                            g2lt[:, 128 * cc:128 * cc + 128],
                            x2s[cc][:, s * MMN:(s + 1) * MMN],
                            start=(cc == 0), stop=(cc == 3))
                x3 = spool.tile([128, BP], F16, tag="x3", bufs=2)
                nc.vector.tensor_mul(x3[:], e34[:, b0:b0 + BP], f2g[:])
                ps3 = ppB.tile([128, BP], F32, tag="ps3", bufs=1)
                for s in range(BP // MMN):
                    nc.tensor.matmul(ps3[:, s * MMN:(s + 1) * MMN], g3lt[:],
                                     x3[:, s * MMN:(s + 1) * MMN],
                                     start=True, stop=True)
                x4 = spool.tile([128, BP], F16, tag="x4", bufs=2)
                nc.vector.tensor_mul(x4[:], e34[:, b0:b0 + BP], ps3[:])
                # psy shares ps3's bank: rows 0..3 (zeros) after X4 reads
                for s in range(BP // MMN):
                    nc.tensor.matmul(ps3[0:4, s * MMN:(s + 1) * MMN], g4lt[:],
                                     x4[:, s * MMN:(s + 1) * MMN],
                                     start=True, stop=True)
                ysb = spool.tile([4, BP], F32, tag="ysb", bufs=2)
                nc.scalar.activation(ysb[:], ps3[0:4, :], COPY)
                nc.sync.dma_start(
                    out=y[:].rearrange("(c i) -> c i", i=CHUNK)[:, b0:b0 + BP],
                    in_=ysb[:])

    nc.compile()
    return nc


def kernel(x, lam0, lam1, pow1, lam2, pow2, lam3, pow3, lam4, pow4):
    x = np.asarray(x, np.float32)
    consts = build_constants(lam0, lam1, pow1, lam2, pow2,
                             lam3, pow3, lam4, pow4)
    nc = build_bass()

    in_maps = []
    for k in range(M_CORES):
        shard = x[k * BS:(k + 1) * BS, :]
        m = {"xt": np.ascontiguousarray(shard.T)}
        m.update(consts)
        in_maps.append(m)

    from concourse.bass_utils import run_bass_kernel_spmd
    res = run_bass_kernel_spmd(nc, in_maps, list(range(M_CORES)))
    out = np.concatenate([res.results[k]["y"] for k in range(M_CORES)])
    return out[:, None].astype(np.float32)


if __name__ == "__main__":
    import reference
    inputs = {k: np.asarray(v) for k, v in reference.setup_inputs().items()}
    got = kernel(**inputs)
    exp = np.asarray(reference.reference(**inputs))
    err = np.abs(got - exp).max() / (np.abs(exp).max() + 1e-30)
    print("shape", got.shape, "relerr", err)
